# revision 61
# baseline (speedup 1.0000x reference)
"""WaveNet-style gated dilated conv layer on 8 Trainium2 NeuronCores.

Strategy: data-parallel over batch (B=8 -> 1 batch element per core).
Per core (batch b):
  z_tanh = sum_k Wc_tanh[k] @ x[:, t-d*(2-k)] + Wcond_tanh @ cond + bias
  z_sig  = likewise for the second half of the 2R conv channels
  h      = tanh(z_tanh) * sigmoid(z_sig)
  out    = W_out @ h, skip = W_skip @ h  (1x1 convs)
All matmuls run in bf16 with fp32 PSUM accumulation.  x and cond are cast
to bf16 on host to halve HBM->SBUF traffic; x is also causal-padded on
host so no on-chip memset is needed.

HBM-traffic/trigger layout (all per core):
 - all weights are packed on host into ONE [128, 1280] bf16 tensor
   (6 conv-tap blocks | 2 cond blocks (rows 0:80) | out | skip), so the
   constant load is a single DMA trigger.
 - out and skip are written bf16 into ONE [128, 2T] DRAM tensor,
   chunk-interleaved ([out_chunk | skip_chunk] per chunk), so each chunk
   flush is a single DMA trigger and output traffic is halved vs fp32.
   Host de-interleaves and casts back to fp32.
 - each engine's dma_start goes to that engine's own DGE queue; one
   queue for everything is itself a bottleneck (~65-79us busy at the
   observed ~240 GB/s) and output flushes starve the input loads queued
   behind them near the tail.  Inputs (x, cond) ride the sync HWDGE
   queue; the weight load and output flushes ride the scalar HWDGE
   queue (gpsimd SWDGE measured slower and is avoided).

The out/skip 1x1 matmuls for tile i are issued during tile i+2's z
matmuls (two-tile software pipelining): h(i) comes out of a scalar
activation + vector multiply chain that trails the PE by most of a
tile, and the PE queue is FIFO past the ldweights window, so issuing
po/ps(i) right after z(i) stalls the PE ~0.4us per tile waiting on h.
Two tiles (~4.3us) of slack also absorb the ~0.6us holes the output
DMA triggers punch in the scalar activation stream at chunk ends.

TRN2 matmul instructions only have room for a single semaphore wait, so
the kernel is structured so no matmul ever needs two: input DMAs are
"observed" by the PE via standalone ldweights instructions before the
first matmul that would otherwise combine a DMA wait with a PSUM WAR
wait.
"""

import sys

for _p in ("/opt/trn_rl_repo",):
    if _p not in sys.path:
        sys.path.append(_p)

from contextlib import ExitStack

import ml_dtypes
import numpy as np

import concourse.bacc as bacc
import concourse.bass as bass
import concourse.tile as tile
from concourse import mybir
from concourse.bass_utils import run_bass_kernel_spmd

B, CIN, T = 8, 128, 16384
R, S, CC, KW = 128, 128, 80, 3
NT = 512           # time-tile width (one PSUM bank of fp32)
N_CORES = 8

BF16 = mybir.dt.bfloat16
FP32 = mybir.dt.float32
FP8 = mybir.dt.float8e4
DR = mybir.MatmulPerfMode.DoubleRow
AF = mybir.ActivationFunctionType

# Packed weights, split into two tensors ordered by first use so the
# first matmul can start after only wts_a + chunk 0 have landed:
#   wts_a: [tan taps k=0,1,2 | cond_tan]            (512 cols, bf16)
#   wts_b: [sig tap k=2 | cond_sig | out | skip]    (512 cols, bf16)
#   w8:    [sig_k0 | sig_k1] DoubleRow lhsT slabs   (256 cols, fp8e4m3)
# Sigmoid taps k=0,1 run as ONE fp8 DoubleRow matmul (K=256 in a single
# pass, 2 rows/cycle): the sigmoid path's error sensitivity is ~4x lower
# than the tanh path's (sigma' <= 1/4 vs tanh' <= 1), and restricting
# fp8 to 2 of the 7 z contractions keeps the measured rel err at
# ~1.2e-2 vs the 2e-2 gate (full-fp8 sigmoid measured 1.73e-2).  All
# sigmoid-path weights are scaled by SIG_SCALE=64 (exact in bf16, and
# it lifts the fp8 weights off the e4m3 subnormal floor); the sigmoid
# activation's scale=1/64 undoes it on PSUM readout.
WA_TAN = 0              # 3 blocks of 128, tanh half
WA_CONDT = 3 * R        # 1 block (rows 0:CC valid)
WTSA_COLS = 4 * R
WB_SIG2 = 0             # sig tap k=2 (bf16, *SIG_SCALE)
WB_CONDS = R            # 1 block (rows 0:CC valid, *SIG_SCALE)
WB_OS = 2 * R           # out block then skip block
WTSB_COLS = 4 * R
W8_COLS = 2 * R
SIG_SCALE = 64.0

_built = {}
_TRACE = False        # set True (e.g. by a test harness) to capture an NTFF profile
_last_results = None  # BassKernelResults of the most recent run


# Streaming chunk widths: ramped at the head so each chunk's input DMA
# (~240 GB/s aggregate) lands before compute catches up to it, large in
# the middle (few DMA triggers), small at the tail (fast final drain).
CHUNK_WIDTHS = [512, 1024, 1536, 2048, 2560, 2560, 2560, 1536, 1024, 512, 512]
assert sum(CHUNK_WIDTHS) == T
CHUNK_STARTS = [sum(CHUNK_WIDTHS[:i]) for i in range(len(CHUNK_WIDTHS))]
NCH = len(CHUNK_WIDTHS)
PREFETCH = 3         # chunk lookahead beyond the current group: mid-kernel
                     # the input queue runs ~zero-margin against compute, so
                     # extra lookahead absorbs its burstiness
WARMUP_MM = 16       # narrow (N=128) cold matmuls covering the initial DMA
                     # latency: each retires in ~130ns so they bridge the
                     # ~2us until chunk-0 data + weights land without delaying
                     # the first real matmul, and keep the PE HAM window busy
                     # so the clock is at 8/8 when real work starts
TAIL_NT = 128        # tile width for the final chunk: the last tile's
                     # act+mul+out/skip+cast drain is exposed at the very end
                     # of the kernel, so make it 4x narrower there

# Output flush units: out/skip for unit [u0, u0+uw) are staged bf16 as
# [out | skip] in SBUF and DMAed to outs[:, 2*u0 : 2*u0+2*uw] as soon as
# the unit's casts land, so the output queue tracks compute with ~1 tile
# of lag instead of draining a chunk-sized backlog after compute ends.
OUT_UNITS = [(j * 1024, 1024) for j in range(15)] + [(15360, 512), (15872, 512)]
assert sum(uw for _, uw in OUT_UNITS) == T


def _build(dilation: int, has_zbias: bool) -> bass.Bass:
    pad = dilation * (KW - 1)

    nc = bacc.Bacc("TRN2", target_bir_lowering=False, debug=False, num_devices=N_CORES)

    x = nc.declare_dram_parameter("x", [CIN, pad + T], BF16, isOutput=False)
    cond = nc.declare_dram_parameter("cond", [CC, T], BF16, isOutput=False)
    # fp8 copies of x for the DoubleRow matmul, packed as interleaved
    # byte PAIRS per time step: xf8[:, j, 0] = x_pad[j] (tap k=0 feed),
    # xf8[:, j, 1] = x_pad[j+dilation] (tap k=1 feed).  Adjacent bytes
    # let the PE fetch both Ko elements of a column in one SBUF read so
    # the DoubleRow matmul streams a column per cycle; with the pair
    # split into two distant slabs it measured 409ns (2 reads/column)
    # instead of ~240ns.
    xf8 = nc.declare_dram_parameter("xf8", [CIN, pad + T, 2], FP8, isOutput=False)
    wtsa = nc.declare_dram_parameter("wtsa", [CIN, WTSA_COLS], BF16, isOutput=False)
    wtsb = nc.declare_dram_parameter("wtsb", [CIN, WTSB_COLS], BF16, isOutput=False)
    w8 = nc.declare_dram_parameter("w8", [CIN, W8_COLS], FP8, isOutput=False)
    if has_zbias:
        zbias = nc.declare_dram_parameter("zbias", [R, 2], FP32, isOutput=False)

    outs = nc.declare_dram_parameter("outs", [R, 2 * T], BF16, isOutput=True)

    with tile.TileContext(nc) as tc, ExitStack() as ctx:
        consts = ctx.enter_context(tc.tile_pool(name="consts", bufs=1))
        inpool = ctx.enter_context(tc.tile_pool(name="inp", bufs=PREFETCH + 2))
        hpool = ctx.enter_context(tc.tile_pool(name="h", bufs=6))
        opool = ctx.enter_context(tc.tile_pool(name="o", bufs=4))
        # PSUM bank budget (8 banks): 3 ztan + 3 zsig + 1 po + 1 ps.  The z
        # accumulators get triple buffering because their reuse is gated on
        # the scalar activation chain, which trails the PE by 1-2 tiles; the
        # out/skip banks recycle against the vector casts, which don't lag.
        zpsum = ctx.enter_context(tc.tile_pool(name="zpsum", bufs=3, space="PSUM"))
        opsum = ctx.enter_context(tc.tile_pool(name="opsum", bufs=1, space="PSUM"))

        # chunk 0 is loaded before anything else: every HWDGE trigger costs
        # ~0.6us of serial sequencer time, so the first-needed data goes first
        xc_tiles = [None] * NCH
        cc_tiles = [None] * NCH

        f8_tiles = [None] * NCH

        def load_chunk(g):
            gs, gw = CHUNK_STARTS[g], CHUNK_WIDTHS[g]
            xc = inpool.tile([CIN, pad + gw], BF16, tag="xc")
            nc.sync.dma_start(xc[:], x[:, gs : gs + pad + gw])
            cc = inpool.tile([CC, gw], BF16, tag="cc")
            nc.sync.dma_start(cc[:], cond[:, gs : gs + gw])
            f8 = inpool.tile([CIN, pad + gw, 2], FP8, tag="f8")
            nc.sync.dma_start(f8[:], xf8[:, gs : gs + pad + gw, :])
            xc_tiles[g], cc_tiles[g], f8_tiles[g] = xc, cc, f8

        wtsa_sb = consts.tile([CIN, WTSA_COLS], BF16)
        nc.sync.dma_start(wtsa_sb[:], wtsa[:])
        load_chunk(0)
        wtsb_sb = consts.tile([CIN, WTSB_COLS], BF16)
        nc.sync.dma_start(wtsb_sb[:], wtsb[:])
        w8_sb = consts.tile([CIN, W8_COLS], FP8)
        nc.sync.dma_start(w8_sb[:], w8[:])
        w8_dr1 = w8_sb[:, 0 : 2 * R].rearrange("p (k m) -> p k m", k=2)
        if has_zbias:
            zbias_sb = consts.tile([R, 2], FP32)
            nc.scalar.dma_start(zbias_sb[:], zbias[:])
            tan_bias = zbias_sb[:, 0:1]
            sig_bias = zbias_sb[:, 1:2]
        else:
            tan_bias = 0.0
            sig_bias = 0.0
        load_chunk(1)

        # Warm-up during the input-load head: narrow matmuls on a zeroed
        # SBUF tile keep the PE HAM busy until real data arrives, and two
        # 1-column activations trigger the tanh/sigmoid table load (~2.7us).
        garbage = consts.tile([CIN, NT], BF16)
        act_sink = consts.tile([R, 1], FP32)
        nc.vector.memset(garbage[:], 0.0)
        nc.vector.memset(act_sink[:], 0.0)
        nc.scalar.activation(act_sink[:], act_sink[:], AF.Tanh, bias=tan_bias)
        nc.scalar.activation(act_sink[:], act_sink[:], AF.Sigmoid, bias=sig_bias)
        for _ in range(WARMUP_MM):
            wz = zpsum.tile([R, NT], FP32, tag="ztan")
            nc.tensor.matmul(
                wz[:, 0:R], garbage[:, 0:R], garbage[:, 0:R], start=True, stop=True
            )

        # pending = [(h, t0, w), ...] tiles whose out/skip matmuls haven't
        # been issued yet (out/skip trail the z matmuls by PIPE tiles).
        # Output staging is unit-based: casts land in the current unit's
        # staging tile, which is flushed the moment it fills.
        PIPE = 2
        pending = []
        ustate = {"idx": 0, "stg": None, "filled": 0}

        def emit_outskip():
            ph, t0, pw = pending.pop(0)
            po = opsum.tile([R, pw], FP32, tag="po")
            nc.tensor.matmul(
                po[:], wtsb_sb[:, WB_OS : WB_OS + R], ph[:], start=True, stop=True
            )
            ps = opsum.tile([S, pw], FP32, tag="ps")
            nc.tensor.matmul(
                ps[:], wtsb_sb[:, WB_OS + R : WB_OS + R + S], ph[:],
                start=True, stop=True,
            )
            u0, uw = OUT_UNITS[ustate["idx"]]
            if ustate["stg"] is None:
                stg_u = opool.tile([R, 2 * uw], BF16, tag="stg")
                ustate["stg"] = stg_u
            stg, off = ustate["stg"], t0 - u0
            nc.vector.tensor_copy(stg[:, off : off + pw], po[:])
            nc.vector.tensor_copy(stg[:, uw + off : uw + off + pw], ps[:])
            ustate["filled"] += pw
            if ustate["filled"] == uw:
                # alternate output units between the scalar HWDGE queue and
                # the gpsimd SWDGE queue: SBUF->HBM writes run at only
                # ~115 GB/s per queue, so one queue (8.4 MB = ~73us) would
                # pace the whole kernel and drain long after compute ends
                eng = nc.scalar if ustate["idx"] % 2 == 0 else nc.gpsimd
                eng.dma_start(outs[:, 2 * u0 : 2 * u0 + 2 * uw], stg[:])
                ustate["idx"] += 1
                ustate["stg"] = None
                ustate["filled"] = 0

        for g in range(NCH):
            gs, gw = CHUNK_STARTS[g], CHUNK_WIDTHS[g]
            for gg in range(g + 1, min(g + PREFETCH + 1, NCH)):
                if xc_tiles[gg] is None:
                    load_chunk(gg)
            xc, cc, f8 = xc_tiles[g], cc_tiles[g], f8_tiles[g]
            # let PE observe the chunk DMA sems on standalone 1-column
            # ldweights (a full-width observer costs a ~430ns PE bubble at
            # every chunk boundary) so no accumulating matmul needs two waits
            nc.tensor.ldweights(xc[:, 0:1])
            nc.tensor.ldweights(cc[:, 0:1])
            nc.tensor.ldweights(f8[:, 0, 0:2])

            nt = TAIL_NT if g == NCH - 1 else NT
            for l0 in range(0, gw, nt):
                w = min(nt, gw - l0)
                ztan = zpsum.tile([R, w], FP32, tag="ztan")
                zsig = zpsum.tile([R, w], FP32, tag="zsig")
                for k in range(KW):
                    xs = xc[:, l0 + dilation * k : l0 + dilation * k + w]
                    nc.tensor.matmul(
                        ztan[:], wtsa_sb[:, WA_TAN + k * R : WA_TAN + (k + 1) * R], xs,
                        start=(k == 0), stop=False,
                    )
                nc.tensor.matmul(
                    ztan[:], wtsa_sb[0:CC, WA_CONDT : WA_CONDT + R], cc[:, l0 : l0 + w],
                    start=False, stop=True,
                )
                # sigmoid half: taps k=0,1 in one fp8 DoubleRow pass, then
                # tap k=2 and cond in bf16 (all weights *SIG_SCALE so the
                # accumulator has one consistent scale)
                nc.tensor.matmul(
                    zsig[:], w8_dr1,
                    f8[:, l0 : l0 + w, :].rearrange("p n k -> p k n"),
                    start=True, stop=False, perf_mode=DR,
                )
                nc.tensor.matmul(
                    zsig[:], wtsb_sb[:, WB_SIG2 : WB_SIG2 + R],
                    xc[:, l0 + 2 * dilation : l0 + 2 * dilation + w],
                    start=False, stop=False,
                )
                nc.tensor.matmul(
                    zsig[:], wtsb_sb[0:CC, WB_CONDS : WB_CONDS + R], cc[:, l0 : l0 + w],
                    start=False, stop=True,
                )

                if len(pending) >= PIPE:
                    emit_outskip()

                th = hpool.tile([R, w], BF16, tag="th")
                nc.scalar.activation(th[:], ztan[:], AF.Tanh, bias=tan_bias)
                sg = hpool.tile([R, w], BF16, tag="sg")
                nc.scalar.activation(
                    sg[:], zsig[:], AF.Sigmoid, bias=sig_bias, scale=1.0 / SIG_SCALE
                )
                # the h multiply runs on the otherwise-idle gpsimd engine:
                # vector is nearly saturated by the two PSUM->SBUF casts per
                # tile (~0.7us each, PSUM reads are single-port on DVE)
                h = hpool.tile([R, w], BF16, tag="h")
                nc.gpsimd.tensor_mul(h[:], th[:], sg[:])
                pending.append((h, gs + l0, w))

        while pending:
            emit_outskip()

    nc.compile()
    return nc


def _pack_weights(w_conv, w_cond, w_out, w_skip):
    bf = ml_dtypes.bfloat16
    f8 = ml_dtypes.float8_e4m3
    wtsa_p = np.zeros((CIN, WTSA_COLS), dtype=bf)
    wtsb_p = np.zeros((CIN, WTSB_COLS), dtype=bf)
    w8_p = np.zeros((CIN, W8_COLS), dtype=f8)
    for k in range(KW):
        wtsa_p[:, WA_TAN + k * R : WA_TAN + (k + 1) * R] = (
            w_conv[0:R, :, k].T.astype(bf)
        )
    wtsa_p[0:CC, WA_CONDT : WA_CONDT + R] = w_cond[0:R, :, 0].T.astype(bf)
    # sigmoid path, all *SIG_SCALE (exact in bf16; lifts fp8 off subnormals)
    ws = (w_conv[R : 2 * R] * SIG_SCALE).astype(np.float32)
    w8_p[:, 0:R] = ws[:, :, 0].T.astype(f8)
    w8_p[:, R : 2 * R] = ws[:, :, 1].T.astype(f8)
    wtsb_p[:, WB_SIG2 : WB_SIG2 + R] = ws[:, :, 2].T.astype(bf)
    wtsb_p[0:CC, WB_CONDS : WB_CONDS + R] = (
        (w_cond[R : 2 * R, :, 0] * SIG_SCALE).T.astype(bf)
    )
    wtsb_p[:, WB_OS : WB_OS + R] = w_out[:, :, 0].T.astype(bf)
    wtsb_p[:, WB_OS + R : WB_OS + R + S] = w_skip[:, :, 0].T.astype(bf)
    return wtsa_p, wtsb_p, w8_p


def kernel(**inputs):
    x = np.asarray(inputs["x"], dtype=np.float32)
    cond = np.asarray(inputs["cond"], dtype=np.float32)
    w_conv = np.asarray(inputs["w_conv"], dtype=np.float32)
    b_conv = np.asarray(inputs["b_conv"], dtype=np.float32)
    w_cond = np.asarray(inputs["w_cond"], dtype=np.float32)
    b_cond = np.asarray(inputs["b_cond"], dtype=np.float32)
    w_out = np.asarray(inputs["w_out"], dtype=np.float32)
    b_out = np.asarray(inputs["b_out"], dtype=np.float32)
    w_skip = np.asarray(inputs["w_skip"], dtype=np.float32)
    b_skip = np.asarray(inputs["b_skip"], dtype=np.float32)
    dilation = int(np.asarray(inputs["dilation"]))
    pad = dilation * (KW - 1)

    zbias_p = np.stack(
        [b_conv[:R] + b_cond[:R], b_conv[R:] + b_cond[R:]], axis=1
    ).astype(np.float32)
    has_zbias = bool(zbias_p.any())

    key = (dilation, has_zbias)
    if key not in _built:
        _built[key] = _build(dilation, has_zbias)
    nc = _built[key]

    wtsa_p, wtsb_p, w8_p = _pack_weights(w_conv, w_cond, w_out, w_skip)
    bf = ml_dtypes.bfloat16
    f8 = ml_dtypes.float8_e4m3
    xb = np.zeros((B, CIN, pad + T), dtype=bf)
    xb[:, :, pad:] = x.astype(bf)
    cb = np.ascontiguousarray(cond.astype(bf))
    # fp8 x as interleaved pairs: [..., j, 0] = x_pad[j], [..., j, 1] =
    # x_pad[j+dilation]
    x8 = np.zeros((B, CIN, pad + T), dtype=f8)
    x8[:, :, pad:] = x.astype(f8)
    xf8b = np.zeros((B, CIN, pad + T, 2), dtype=f8)
    xf8b[:, :, :, 0] = x8
    xf8b[:, :, : pad + T - dilation, 1] = x8[:, :, dilation:]

    in_maps = []
    for b in range(B):
        m = {
            "x": xb[b], "cond": cb[b], "xf8": xf8b[b],
            "wtsa": wtsa_p, "wtsb": wtsb_p, "w8": w8_p,
        }
        if has_zbias:
            m["zbias"] = zbias_p
        in_maps.append(m)
    br = run_bass_kernel_spmd(nc, in_maps, list(range(N_CORES)), trace=_TRACE)
    global _last_results
    _last_results = br
    res = br.results
    output = np.empty((B, R, T), dtype=np.float32)
    skip = np.empty((B, S, T), dtype=np.float32)
    for b in range(B):
        ob = np.asarray(res[b]["outs"])
        for u0, uw in OUT_UNITS:
            output[b, :, u0 : u0 + uw] = ob[:, 2 * u0 : 2 * u0 + uw]
            skip[b, :, u0 : u0 + uw] = ob[:, 2 * u0 + uw : 2 * u0 + 2 * uw]
    if b_out.any():
        output = output + b_out[None, :, None]
    if b_skip.any():
        skip = skip + b_skip[None, :, None]
    return (output, skip)


# revision 62
# speedup vs baseline: 1.0408x; 1.0408x over previous
"""WaveNet-style gated dilated conv layer on 8 Trainium2 NeuronCores.

Strategy: data-parallel over batch (B=8 -> 1 batch element per core).
Per core (batch b):
  z_tanh = sum_k Wc_tanh[k] @ x[:, t-d*(2-k)] + Wcond_tanh @ cond + bias
  z_sig  = likewise for the second half of the 2R conv channels
  h      = tanh(z_tanh) * sigmoid(z_sig)
  out    = W_out @ h, skip = W_skip @ h  (1x1 convs)
All matmuls run in bf16 with fp32 PSUM accumulation.  x and cond are cast
to bf16 on host to halve HBM->SBUF traffic; x is also causal-padded on
host so no on-chip memset is needed.

HBM-traffic/trigger layout (all per core):
 - all weights are packed on host into ONE [128, 1280] bf16 tensor
   (6 conv-tap blocks | 2 cond blocks (rows 0:80) | out | skip), so the
   constant load is a single DMA trigger.
 - out and skip are written bf16 into ONE [128, 2T] DRAM tensor,
   chunk-interleaved ([out_chunk | skip_chunk] per chunk), so each chunk
   flush is a single DMA trigger and output traffic is halved vs fp32.
   Host de-interleaves and casts back to fp32.
 - each engine's dma_start goes to that engine's own DGE queue; one
   queue for everything is itself a bottleneck (~65-79us busy at the
   observed ~240 GB/s) and output flushes starve the input loads queued
   behind them near the tail.  Inputs (x, cond) ride the sync HWDGE
   queue; the weight load and output flushes ride the scalar HWDGE
   queue (gpsimd SWDGE measured slower and is avoided).

The out/skip 1x1 matmuls for tile i are issued during tile i+2's z
matmuls (two-tile software pipelining): h(i) comes out of a scalar
activation + vector multiply chain that trails the PE by most of a
tile, and the PE queue is FIFO past the ldweights window, so issuing
po/ps(i) right after z(i) stalls the PE ~0.4us per tile waiting on h.
Two tiles (~4.3us) of slack also absorb the ~0.6us holes the output
DMA triggers punch in the scalar activation stream at chunk ends.

TRN2 matmul instructions only have room for a single semaphore wait, so
the kernel is structured so no matmul ever needs two: input DMAs are
"observed" by the PE via standalone ldweights instructions before the
first matmul that would otherwise combine a DMA wait with a PSUM WAR
wait.
"""

import sys

for _p in ("/opt/trn_rl_repo",):
    if _p not in sys.path:
        sys.path.append(_p)

from contextlib import ExitStack

import ml_dtypes
import numpy as np

import concourse.bacc as bacc
import concourse.bass as bass
import concourse.tile as tile
from concourse import mybir
from concourse.bass_utils import run_bass_kernel_spmd

B, CIN, T = 8, 128, 16384
R, S, CC, KW = 128, 128, 80, 3
NT = 512           # time-tile width (one PSUM bank of fp32)
N_CORES = 8

BF16 = mybir.dt.bfloat16
FP32 = mybir.dt.float32
FP8 = mybir.dt.float8e4
DR = mybir.MatmulPerfMode.DoubleRow
AF = mybir.ActivationFunctionType

# Packed weights, split into two tensors ordered by first use so the
# first matmul can start after only wts_a + chunk 0 have landed:
#   wts_a: [tan taps k=0,1,2 | cond_tan]            (512 cols, bf16)
#   wts_b: [sig tap k=2 | cond_sig | out | skip]    (512 cols, bf16)
#   w8:    [sig_k0 | sig_k1] DoubleRow lhsT slabs   (256 cols, fp8e4m3)
# Sigmoid taps k=0,1 run as ONE fp8 DoubleRow matmul (K=256 in a single
# pass, 2 rows/cycle): the sigmoid path's error sensitivity is ~4x lower
# than the tanh path's (sigma' <= 1/4 vs tanh' <= 1), and restricting
# fp8 to 2 of the 7 z contractions keeps the measured rel err at
# ~1.2e-2 vs the 2e-2 gate (full-fp8 sigmoid measured 1.73e-2).  All
# sigmoid-path weights are scaled by SIG_SCALE=64 (exact in bf16, and
# it lifts the fp8 weights off the e4m3 subnormal floor); the sigmoid
# activation's scale=1/64 undoes it on PSUM readout.
WA_TAN = 0              # 3 blocks of 128, tanh half
WA_CONDT = 3 * R        # 1 block (rows 0:CC valid)
WTSA_COLS = 4 * R
WB_SIG2 = 0             # sig tap k=2 (bf16, *SIG_SCALE)
WB_CONDS = R            # 1 block (rows 0:CC valid, *SIG_SCALE)
WB_OS = 2 * R           # out block then skip block
WTSB_COLS = 4 * R
W8_COLS = 2 * R
SIG_SCALE = 64.0

_built = {}
_TRACE = False        # set True (e.g. by a test harness) to capture an NTFF profile
_last_results = None  # BassKernelResults of the most recent run


# Streaming chunk widths: ramped at the head so each chunk's input DMA
# (~240 GB/s aggregate) lands before compute catches up to it, large in
# the middle (few DMA triggers), small at the tail (fast final drain).
CHUNK_WIDTHS = [512, 1024, 1536, 2048, 2560, 2560, 2560, 1536, 1024, 512, 512]
assert sum(CHUNK_WIDTHS) == T
CHUNK_STARTS = [sum(CHUNK_WIDTHS[:i]) for i in range(len(CHUNK_WIDTHS))]
NCH = len(CHUNK_WIDTHS)
PREFETCH = 3         # chunk lookahead beyond the current group: mid-kernel
                     # the input queue runs ~zero-margin against compute, so
                     # extra lookahead absorbs its burstiness
WARMUP_MM = 28       # narrow (N=128) cold matmuls covering the initial DMA
                     # latency: each retires in ~130ns so they bridge the
                     # ~2us until chunk-0 data + weights land without delaying
                     # the first real matmul, and keep the PE HAM window busy
                     # so the clock is at 8/8 when real work starts
TAIL_NT = 128        # tile width for the final chunk: the last tile's
                     # act+mul+out/skip+cast drain is exposed at the very end
                     # of the kernel, so make it 4x narrower there

# Output flush units: out/skip for unit [u0, u0+uw) are staged bf16 as
# [out | skip] in SBUF and DMAed to outs[:, 2*u0 : 2*u0+2*uw] as soon as
# the unit's casts land, so the output queue tracks compute with ~1 tile
# of lag instead of draining a chunk-sized backlog after compute ends.
OUT_UNITS = [(j * 1024, 1024) for j in range(15)] + [(15360, 512), (15872, 512)]
assert sum(uw for _, uw in OUT_UNITS) == T


def _build(dilation: int, has_zbias: bool) -> bass.Bass:
    pad = dilation * (KW - 1)

    nc = bacc.Bacc("TRN2", target_bir_lowering=False, debug=False, num_devices=N_CORES)

    x = nc.declare_dram_parameter("x", [CIN, pad + T], BF16, isOutput=False)
    cond = nc.declare_dram_parameter("cond", [CC, T], BF16, isOutput=False)
    # fp8 copies of x for the DoubleRow matmul, packed as interleaved
    # byte PAIRS per time step: xf8[:, j, 0] = x_pad[j] (tap k=0 feed),
    # xf8[:, j, 1] = x_pad[j+dilation] (tap k=1 feed).  Adjacent bytes
    # let the PE fetch both Ko elements of a column in one SBUF read so
    # the DoubleRow matmul streams a column per cycle; with the pair
    # split into two distant slabs it measured 409ns (2 reads/column)
    # instead of ~240ns.
    xf8 = nc.declare_dram_parameter("xf8", [CIN, pad + T, 2], FP8, isOutput=False)
    wtsa = nc.declare_dram_parameter("wtsa", [CIN, WTSA_COLS], BF16, isOutput=False)
    wtsb = nc.declare_dram_parameter("wtsb", [CIN, WTSB_COLS], BF16, isOutput=False)
    w8 = nc.declare_dram_parameter("w8", [CIN, W8_COLS], FP8, isOutput=False)
    if has_zbias:
        zbias = nc.declare_dram_parameter("zbias", [R, 2], FP32, isOutput=False)

    outs = nc.declare_dram_parameter("outs", [R, 2 * T], BF16, isOutput=True)

    with tile.TileContext(nc) as tc, ExitStack() as ctx:
        consts = ctx.enter_context(tc.tile_pool(name="consts", bufs=1))
        inpool = ctx.enter_context(tc.tile_pool(name="inp", bufs=PREFETCH + 2))
        hpool = ctx.enter_context(tc.tile_pool(name="h", bufs=6))
        opool = ctx.enter_context(tc.tile_pool(name="o", bufs=4))
        # PSUM bank budget (8 banks): 3 ztan + 3 zsig + 1 po + 1 ps.  The z
        # accumulators get triple buffering because their reuse is gated on
        # the scalar activation chain, which trails the PE by 1-2 tiles; the
        # out/skip banks recycle against the vector casts, which don't lag.
        zpsum = ctx.enter_context(tc.tile_pool(name="zpsum", bufs=3, space="PSUM"))
        opsum = ctx.enter_context(tc.tile_pool(name="opsum", bufs=1, space="PSUM"))

        # chunk 0 is loaded before anything else: every HWDGE trigger costs
        # ~0.6us of serial sequencer time, so the first-needed data goes first
        xc_tiles = [None] * NCH
        cc_tiles = [None] * NCH

        f8_tiles = [None] * NCH

        def load_chunk(g):
            gs, gw = CHUNK_STARTS[g], CHUNK_WIDTHS[g]
            xc = inpool.tile([CIN, pad + gw], BF16, tag="xc")
            nc.sync.dma_start(xc[:], x[:, gs : gs + pad + gw])
            cc = inpool.tile([CC, gw], BF16, tag="cc")
            nc.sync.dma_start(cc[:], cond[:, gs : gs + gw])
            f8 = inpool.tile([CIN, pad + gw, 2], FP8, tag="f8")
            nc.sync.dma_start(f8[:], xf8[:, gs : gs + pad + gw, :])
            xc_tiles[g], cc_tiles[g], f8_tiles[g] = xc, cc, f8

        wtsa_sb = consts.tile([CIN, WTSA_COLS], BF16)
        nc.sync.dma_start(wtsa_sb[:], wtsa[:])
        load_chunk(0)
        wtsb_sb = consts.tile([CIN, WTSB_COLS], BF16)
        nc.sync.dma_start(wtsb_sb[:], wtsb[:])
        w8_sb = consts.tile([CIN, W8_COLS], FP8)
        nc.sync.dma_start(w8_sb[:], w8[:])
        w8_dr1 = w8_sb[:, 0 : 2 * R].rearrange("p (k m) -> p k m", k=2)
        if has_zbias:
            zbias_sb = consts.tile([R, 2], FP32)
            nc.scalar.dma_start(zbias_sb[:], zbias[:])
            tan_bias = zbias_sb[:, 0:1]
            sig_bias = zbias_sb[:, 1:2]
        else:
            tan_bias = 0.0
            sig_bias = 0.0
        load_chunk(1)

        # Warm-up during the input-load head: narrow matmuls on a zeroed
        # SBUF tile keep the PE HAM busy until real data arrives, and two
        # 1-column activations trigger the tanh/sigmoid table load (~2.7us).
        garbage = consts.tile([CIN, NT], BF16)
        act_sink = consts.tile([R, 1], FP32)
        nc.vector.memset(garbage[:], 0.0)
        nc.vector.memset(act_sink[:], 0.0)
        nc.scalar.activation(act_sink[:], act_sink[:], AF.Tanh, bias=tan_bias)
        nc.scalar.activation(act_sink[:], act_sink[:], AF.Sigmoid, bias=sig_bias)
        for _ in range(WARMUP_MM):
            wz = zpsum.tile([R, NT], FP32, tag="ztan")
            nc.tensor.matmul(
                wz[:, 0:R], garbage[:, 0:R], garbage[:, 0:R], start=True, stop=True
            )

        # pending = [(h, t0, w), ...] tiles whose out/skip matmuls haven't
        # been issued yet (out/skip trail the z matmuls by PIPE tiles).
        # Output staging is unit-based: casts land in the current unit's
        # staging tile, which is flushed the moment it fills.
        PIPE = 2
        pending = []
        ustate = {"idx": 0, "stg": None, "filled": 0}

        def emit_outskip():
            ph, t0, pw = pending.pop(0)
            po = opsum.tile([R, pw], FP32, tag="po")
            nc.tensor.matmul(
                po[:], wtsb_sb[:, WB_OS : WB_OS + R], ph[:], start=True, stop=True
            )
            ps = opsum.tile([S, pw], FP32, tag="ps")
            nc.tensor.matmul(
                ps[:], wtsb_sb[:, WB_OS + R : WB_OS + R + S], ph[:],
                start=True, stop=True,
            )
            u0, uw = OUT_UNITS[ustate["idx"]]
            if ustate["stg"] is None:
                stg_u = opool.tile([R, 2 * uw], BF16, tag="stg")
                ustate["stg"] = stg_u
            stg, off = ustate["stg"], t0 - u0
            nc.vector.tensor_copy(stg[:, off : off + pw], po[:])
            nc.vector.tensor_copy(stg[:, uw + off : uw + off + pw], ps[:])
            ustate["filled"] += pw
            if ustate["filled"] == uw:
                # alternate output units between the scalar HWDGE queue and
                # the gpsimd SWDGE queue: SBUF->HBM writes run at only
                # ~115 GB/s per queue, so one queue (8.4 MB = ~73us) would
                # pace the whole kernel and drain long after compute ends
                eng = nc.scalar if ustate["idx"] % 2 == 0 else nc.gpsimd
                eng.dma_start(outs[:, 2 * u0 : 2 * u0 + 2 * uw], stg[:])
                ustate["idx"] += 1
                ustate["stg"] = None
                ustate["filled"] = 0

        for g in range(NCH):
            gs, gw = CHUNK_STARTS[g], CHUNK_WIDTHS[g]
            for gg in range(g + 1, min(g + PREFETCH + 1, NCH)):
                if xc_tiles[gg] is None:
                    load_chunk(gg)
            xc, cc, f8 = xc_tiles[g], cc_tiles[g], f8_tiles[g]
            # let PE observe the chunk DMA sems on standalone 1-column
            # ldweights (a full-width observer costs a ~430ns PE bubble at
            # every chunk boundary) so no accumulating matmul needs two waits
            nc.tensor.ldweights(xc[:, 0:1])
            nc.tensor.ldweights(cc[:, 0:1])
            nc.tensor.ldweights(f8[:, 0, 0:2])

            nt = TAIL_NT if g == NCH - 1 else NT
            for l0 in range(0, gw, nt):
                w = min(nt, gw - l0)
                ztan = zpsum.tile([R, w], FP32, tag="ztan")
                zsig = zpsum.tile([R, w], FP32, tag="zsig")
                for k in range(KW):
                    xs = xc[:, l0 + dilation * k : l0 + dilation * k + w]
                    nc.tensor.matmul(
                        ztan[:], wtsa_sb[:, WA_TAN + k * R : WA_TAN + (k + 1) * R], xs,
                        start=(k == 0), stop=False,
                    )
                nc.tensor.matmul(
                    ztan[:], wtsa_sb[0:CC, WA_CONDT : WA_CONDT + R], cc[:, l0 : l0 + w],
                    start=False, stop=True,
                )
                # sigmoid half: taps k=0,1 in one fp8 DoubleRow pass, then
                # tap k=2 and cond in bf16 (all weights *SIG_SCALE so the
                # accumulator has one consistent scale)
                nc.tensor.matmul(
                    zsig[:], w8_dr1,
                    f8[:, l0 : l0 + w, :].rearrange("p n k -> p k n"),
                    start=True, stop=False, perf_mode=DR,
                )
                nc.tensor.matmul(
                    zsig[:], wtsb_sb[:, WB_SIG2 : WB_SIG2 + R],
                    xc[:, l0 + 2 * dilation : l0 + 2 * dilation + w],
                    start=False, stop=False,
                )
                nc.tensor.matmul(
                    zsig[:], wtsb_sb[0:CC, WB_CONDS : WB_CONDS + R], cc[:, l0 : l0 + w],
                    start=False, stop=True,
                )

                if len(pending) >= PIPE:
                    emit_outskip()

                th = hpool.tile([R, w], BF16, tag="th")
                nc.scalar.activation(th[:], ztan[:], AF.Tanh, bias=tan_bias)
                sg = hpool.tile([R, w], BF16, tag="sg")
                nc.scalar.activation(
                    sg[:], zsig[:], AF.Sigmoid, bias=sig_bias, scale=1.0 / SIG_SCALE
                )
                # the h multiply runs on the otherwise-idle gpsimd engine:
                # vector is nearly saturated by the two PSUM->SBUF casts per
                # tile (~0.7us each, PSUM reads are single-port on DVE)
                h = hpool.tile([R, w], BF16, tag="h")
                nc.gpsimd.tensor_mul(h[:], th[:], sg[:])
                pending.append((h, gs + l0, w))

        while pending:
            emit_outskip()

    nc.compile()
    return nc


def _pack_weights(w_conv, w_cond, w_out, w_skip):
    bf = ml_dtypes.bfloat16
    f8 = ml_dtypes.float8_e4m3
    wtsa_p = np.zeros((CIN, WTSA_COLS), dtype=bf)
    wtsb_p = np.zeros((CIN, WTSB_COLS), dtype=bf)
    w8_p = np.zeros((CIN, W8_COLS), dtype=f8)
    for k in range(KW):
        wtsa_p[:, WA_TAN + k * R : WA_TAN + (k + 1) * R] = (
            w_conv[0:R, :, k].T.astype(bf)
        )
    wtsa_p[0:CC, WA_CONDT : WA_CONDT + R] = w_cond[0:R, :, 0].T.astype(bf)
    # sigmoid path, all *SIG_SCALE (exact in bf16; lifts fp8 off subnormals)
    ws = (w_conv[R : 2 * R] * SIG_SCALE).astype(np.float32)
    w8_p[:, 0:R] = ws[:, :, 0].T.astype(f8)
    w8_p[:, R : 2 * R] = ws[:, :, 1].T.astype(f8)
    wtsb_p[:, WB_SIG2 : WB_SIG2 + R] = ws[:, :, 2].T.astype(bf)
    wtsb_p[0:CC, WB_CONDS : WB_CONDS + R] = (
        (w_cond[R : 2 * R, :, 0] * SIG_SCALE).T.astype(bf)
    )
    wtsb_p[:, WB_OS : WB_OS + R] = w_out[:, :, 0].T.astype(bf)
    wtsb_p[:, WB_OS + R : WB_OS + R + S] = w_skip[:, :, 0].T.astype(bf)
    return wtsa_p, wtsb_p, w8_p


def kernel(**inputs):
    x = np.asarray(inputs["x"], dtype=np.float32)
    cond = np.asarray(inputs["cond"], dtype=np.float32)
    w_conv = np.asarray(inputs["w_conv"], dtype=np.float32)
    b_conv = np.asarray(inputs["b_conv"], dtype=np.float32)
    w_cond = np.asarray(inputs["w_cond"], dtype=np.float32)
    b_cond = np.asarray(inputs["b_cond"], dtype=np.float32)
    w_out = np.asarray(inputs["w_out"], dtype=np.float32)
    b_out = np.asarray(inputs["b_out"], dtype=np.float32)
    w_skip = np.asarray(inputs["w_skip"], dtype=np.float32)
    b_skip = np.asarray(inputs["b_skip"], dtype=np.float32)
    dilation = int(np.asarray(inputs["dilation"]))
    pad = dilation * (KW - 1)

    zbias_p = np.stack(
        [b_conv[:R] + b_cond[:R], b_conv[R:] + b_cond[R:]], axis=1
    ).astype(np.float32)
    has_zbias = bool(zbias_p.any())

    key = (dilation, has_zbias)
    if key not in _built:
        _built[key] = _build(dilation, has_zbias)
    nc = _built[key]

    wtsa_p, wtsb_p, w8_p = _pack_weights(w_conv, w_cond, w_out, w_skip)
    bf = ml_dtypes.bfloat16
    f8 = ml_dtypes.float8_e4m3
    xb = np.zeros((B, CIN, pad + T), dtype=bf)
    xb[:, :, pad:] = x.astype(bf)
    cb = np.ascontiguousarray(cond.astype(bf))
    # fp8 x as interleaved pairs: [..., j, 0] = x_pad[j], [..., j, 1] =
    # x_pad[j+dilation]
    x8 = np.zeros((B, CIN, pad + T), dtype=f8)
    x8[:, :, pad:] = x.astype(f8)
    xf8b = np.zeros((B, CIN, pad + T, 2), dtype=f8)
    xf8b[:, :, :, 0] = x8
    xf8b[:, :, : pad + T - dilation, 1] = x8[:, :, dilation:]

    in_maps = []
    for b in range(B):
        m = {
            "x": xb[b], "cond": cb[b], "xf8": xf8b[b],
            "wtsa": wtsa_p, "wtsb": wtsb_p, "w8": w8_p,
        }
        if has_zbias:
            m["zbias"] = zbias_p
        in_maps.append(m)
    br = run_bass_kernel_spmd(nc, in_maps, list(range(N_CORES)), trace=_TRACE)
    global _last_results
    _last_results = br
    res = br.results
    output = np.empty((B, R, T), dtype=np.float32)
    skip = np.empty((B, S, T), dtype=np.float32)
    for b in range(B):
        ob = np.asarray(res[b]["outs"])
        for u0, uw in OUT_UNITS:
            output[b, :, u0 : u0 + uw] = ob[:, 2 * u0 : 2 * u0 + uw]
            skip[b, :, u0 : u0 + uw] = ob[:, 2 * u0 + uw : 2 * u0 + 2 * uw]
    if b_out.any():
        output = output + b_out[None, :, None]
    if b_skip.any():
        skip = skip + b_skip[None, :, None]
    return (output, skip)
